# revision 25
# baseline (speedup 1.0000x reference)
"""Trainium2 Bass kernel for nn_PixelAggregationNetwork.

Strategy (8 NeuronCores, memory-bound):
  x is [B=4, C=32, H=512, W=500] f32 (~131 MB). All downstream math
  (tree/LCA/loss) operates on tiny per-segment reductions of x, so the
  kernel's only real job is one streaming pass over x.

  Shard along H: core k owns rows [64k, 64k+64) for all (b, c), viewed as
  [B*C = 128 partitions, 64*500]. Per 8-row chunk:
    - DMA chunk -> SBUF
    - VectorE tensor_reduce (axis=XY) -> per-strip sums [128, 10]
    - TensorE matmul with a [128, 4] block mask (1/32, f32r) -> per-pixel
      channel means ("gray") [4, 500] per row in PSUM -> DMA to DRAM
  Core outputs: segment-sum partials [128, 10] f32 and gray rows [4, 64*500]
  f16. Host combines partials (f64) and finishes the 21-node hierarchy + loss.
"""

import numpy as np
from contextlib import ExitStack

import concourse.bass as bass
import concourse.mybir as mybir
import concourse.tile as tile
from concourse.bass_utils import run_bass_kernel_spmd

B, C, H, W = 4, 32, 512, 500
S = 10
SW = W // S
TW = 0.5
MARGIN = 1.0
REG_W = 0.01

NCORES = 8
HC = H // NCORES          # 64 rows per core
R = 8                     # rows per chunk
NCH = HC // R             # chunks per core

F32 = mybir.dt.float32
F16 = mybir.dt.float16


# ---------------------------------------------------------------- tree/LCA
def _build_tree():
    sizes = []
    n = S
    while True:
        sizes.append(n)
        if n == 1:
            break
        n = (n + 1) // 2
    offs = np.cumsum([0] + sizes)
    total = int(offs[-1])
    parent = np.arange(total)
    level = np.zeros(total, np.int32)
    for l, sz in enumerate(sizes):
        for i in range(sz):
            g = offs[l] + i
            level[g] = l
            if l + 1 < len(sizes):
                parent[g] = offs[l + 1] + i // 2
    L = len(sizes)
    chain = np.zeros((total, L), np.int64)
    for g in range(total):
        for l in range(L):
            if l < level[g]:
                chain[g, l] = -1 - g
            else:
                a = g
                while level[a] < l:
                    a = int(parent[a])
                chain[g, l] = a
    return sizes, parent.astype(np.int32), level, chain


SIZES, PARENT, LEVEL, CHAIN = _build_tree()
MAXL = len(SIZES) - 1
NTOT = PARENT.shape[0]


def _lca_matrix():
    eq = CHAIN[:, None, :] == CHAIN[None, :, :]
    first = np.argmax(eq, axis=-1)
    return CHAIN[np.arange(NTOT)[:, None], first].astype(np.int32)


LCA = _lca_matrix()


# ---------------------------------------------------------------- device program
_PROGRAM = None


def _build_program():
    nc = bass.Bass(trn_type="TRN2", num_swdge_queues=4)
    xs = nc.declare_dram_parameter("xs", [B * C, HC * W], F32, isOutput=False)
    mask = nc.declare_dram_parameter("mask", [B * C, B], F16, isOutput=False)
    seg_out = nc.declare_dram_parameter("seg_out", [B * C, S], F32, isOutput=True)
    # gray ships in the PSUM-native permutation [pixel%125, (half, j, b)];
    # the host undoes it (gray[b, (half*128+j)*125 + p]).
    gray_out = nc.declare_dram_parameter("gray_out", [125, 1024], F16, isOutput=True)

    # Raw-Bass program (no TileContext): the walrus build in this container
    # supports only ONE embedded sync-wait per DMA/matmul/drain instruction,
    # which Tile's auto-generated semaphores and kernel-tail drain violate
    # structurally. With explicit semaphores every instruction carries at
    # most one wait:
    #   Pool : 9 casting SWDGE DMAs (f32->f16), inc in_sem by 16 each
    #   PE   : per chunk, 1 wait on in_sem, then 32 "flipped" matmuls
    #          (stationary = 125-pixel slice, moving = [128,4] mask,
    #          out = [125 pixels, 4 batches]); 256 outputs fill exactly
    #          2 PSUM banks, inc pe_sem after each chunk
    #   DVE  : per chunk, 1 wait on in_sem, strip-sum tensor_reduce
    #   ACT  : waits pe_sem, evacuates each full PSUM bank (f32->f16),
    #          issues the gray HWDGE DMA itself
    #   SP   : waits dve_sem, issues the seg DMA
    PJ = 125                   # pixels per matmul (stationary free dim)
    JPC = R * W // PJ          # matmuls per chunk
    JPB = 512 // B             # matmul outputs per PSUM bank (128)
    CPB = JPB // JPC           # chunks per PSUM bank

    with ExitStack() as ctx:
        t16 = ctx.enter_context(nc.sbuf_tensor([B * C, HC * W], F16))
        mask_t = ctx.enter_context(nc.sbuf_tensor([B * C, B], F16))
        segbuf = ctx.enter_context(nc.sbuf_tensor([B * C, NCH * S], F32))
        seg_final = ctx.enter_context(nc.sbuf_tensor([B * C, S], F32))
        gbuf = ctx.enter_context(nc.sbuf_tensor([PJ, 2 * JPB * B], F16))
        ps0 = ctx.enter_context(nc.psum_tensor([B * C, JPB * B], F32))
        ps1 = ctx.enter_context(nc.psum_tensor([B * C, JPB * B], F32))
        in_sem = ctx.enter_context(nc.semaphore("in_sem"))
        pe_sem = ctx.enter_context(nc.semaphore("pe_sem"))
        dve_sem = ctx.enter_context(nc.semaphore("dve_sem"))
        g_sem = ctx.enter_context(nc.semaphore("g_sem"))
        s_sem = ctx.enter_context(nc.semaphore("s_sem"))
        block = ctx.enter_context(nc.Block())
        banks = [ps0, ps1]

        @block.gpsimd
        def _(pool):
            pool.dma_start(mask_t[:], mask[:]).then_inc(in_sem, 16)
            for ch in range(NCH):
                sl = slice(ch * R * W, (ch + 1) * R * W)
                pool.dma_start(t16[:, sl], xs[:, sl]).then_inc(in_sem, 16)

        @block.tensor
        def _(pe):
            for ch in range(NCH):
                pe.wait_ge(in_sem, 16 * (ch + 2))
                for jj in range(JPC):
                    j = ch * JPC + jj
                    pst = banks[j // JPB]
                    slot = j % JPB
                    mm = pe.matmul(
                        pst[0:PJ, slot * B:(slot + 1) * B],
                        t16[:, j * PJ:(j + 1) * PJ],
                        mask_t[:],
                        start=True, stop=True,
                    )
                mm.then_inc(pe_sem, 1)

        @block.vector
        def _(dve):
            for ch in range(NCH):
                dve.wait_ge(in_sem, 16 * (ch + 2))
                v = t16[:, ch * R * W:(ch + 1) * R * W].rearrange(
                    "p (r s w) -> p s r w", r=R, s=S, w=SW
                )
                dve.tensor_reduce(
                    segbuf[:, ch * S:(ch + 1) * S], v,
                    axis=mybir.AxisListType.XY, op=mybir.AluOpType.add,
                )
            dve.tensor_reduce(
                seg_final[:],
                segbuf[:].rearrange("p (c s) -> p s c", c=NCH, s=S),
                axis=mybir.AxisListType.X, op=mybir.AluOpType.add,
            ).then_inc(dve_sem, 1)

        @block.scalar
        def _(act):
            for half in range(2):
                act.wait_ge(pe_sem, CPB * (half + 1))
                act.copy(
                    gbuf[:, half * JPB * B:(half + 1) * JPB * B],
                    banks[half][0:PJ, :],
                )
            act.dma_start(gray_out[:], gbuf[:]).then_inc(g_sem, 16)
            act.wait_ge(g_sem, 16)

        @block.sync
        def _(sp):
            sp.wait_ge(dve_sem, 1)
            sp.dma_start(seg_out[:], seg_final[:]).then_inc(s_sem, 16)
            sp.wait_ge(s_sem, 16)
    return nc


def _get_program():
    global _PROGRAM
    if _PROGRAM is None:
        _PROGRAM = _build_program()
    return _PROGRAM


def _make_mask():
    m = np.zeros((B * C, B), np.float16)
    m[np.arange(B * C), np.arange(B * C) // C] = 1.0 / C
    return m


def _run_device(x, **kwargs):
    nc = _get_program()
    mask_np = _make_mask()
    in_maps = []
    for k in range(NCORES):
        xs = np.ascontiguousarray(
            x[:, :, k * HC:(k + 1) * HC, :].reshape(B * C, HC * W)
        )
        in_maps.append({"xs": xs, "mask": mask_np})
    return run_bass_kernel_spmd(nc, in_maps, list(range(NCORES)), **kwargs)


def _finalize(seg, gray):
    """seg: [B*C, S] f64 total strip sums; gray: [B, H, W] f64 channel means."""
    nodes = (seg / (H * SW)).reshape(B, C, S).transpose(0, 2, 1)      # [B,S,C]
    texture = (gray ** 2).reshape(B, H, S, SW).sum(axis=(1, 3))      # [B,S]
    feats = nodes * (1.0 - TW) + texture[..., None] * TW             # [B,S,C]

    reg = 0.0
    cur = feats
    for sz in SIZES[1:]:
        n = cur.shape[1]
        ids = np.arange(n) // 2
        counts = np.bincount(ids, minlength=sz).astype(np.float64)
        summed = np.zeros((sz, B, C), np.float64)
        np.add.at(summed, ids, cur.transpose(1, 0, 2))
        cur = (summed / counts[:, None, None]).transpose(1, 0, 2)
        reg += (cur ** 2).mean()

    diff = feats[:, :, None, :] - feats[:, None, :, :]
    d = np.sqrt((diff ** 2).sum(-1) + 1e-12)                          # [B,S,S]
    w = 1.0 - LEVEL[LCA[:S, :S]].astype(np.float64) / MAXL
    loss = (w * d ** 2 + (1.0 - w) * np.maximum(MARGIN - d, 0.0) ** 2).mean() \
        + REG_W * reg

    tree = np.broadcast_to(
        np.stack([PARENT, LEVEL], axis=-1).astype(np.int32)[None], (B, NTOT, 2)
    ).copy()
    return tree, np.float32(loss)


def _gather(res):
    seg = np.zeros((B * C, S), np.float64)
    gray = np.zeros((B, H, W), np.float64)
    for k in range(NCORES):
        seg += res.results[k]["seg_out"].astype(np.float64)
        g = res.results[k]["gray_out"].astype(np.float64)      # [125, 1024]
        g = g.reshape(125, 2, 128, B).transpose(3, 1, 2, 0)    # [B, half, j, p]
        gray[:, k * HC:(k + 1) * HC, :] = g.reshape(B, HC, W)
    return seg, gray


def kernel(x):
    x = np.asarray(x, dtype=np.float32)
    res = _run_device(x)
    seg, gray = _gather(res)
    return _finalize(seg, gray)


# revision 26
# speedup vs baseline: 1.1159x; 1.1159x over previous
"""Trainium2 Bass kernel for nn_PixelAggregationNetwork.

Strategy (8 NeuronCores, memory-bound):
  x is [B=4, C=32, H=512, W=500] f32 (~131 MB). All downstream math
  (tree/LCA/loss) operates on tiny per-segment reductions of x, so the
  kernel's only real job is one streaming pass over x.

  Shard along H: core k owns rows [64k, 64k+64) for all (b, c), viewed as
  [B*C = 128 partitions, 64*500]. Per 8-row chunk:
    - DMA chunk -> SBUF
    - VectorE tensor_reduce (axis=XY) -> per-strip sums [128, 10]
    - TensorE matmul with a [128, 4] block mask (1/32, f32r) -> per-pixel
      channel means ("gray") [4, 500] per row in PSUM -> DMA to DRAM
  Core outputs: segment-sum partials [128, 10] f32 and gray rows [4, 64*500]
  f16. Host combines partials (f64) and finishes the 21-node hierarchy + loss.
"""

import numpy as np
from contextlib import ExitStack

import concourse.bass as bass
import concourse.mybir as mybir
import concourse.tile as tile
from concourse.bass_utils import run_bass_kernel_spmd

B, C, H, W = 4, 32, 512, 500
S = 10
SW = W // S
TW = 0.5
MARGIN = 1.0
REG_W = 0.01

NCORES = 8
HC = H // NCORES          # 64 rows per core
R = 4                     # rows per chunk
NCH = HC // R             # chunks per core

F32 = mybir.dt.float32
F16 = mybir.dt.float16


# ---------------------------------------------------------------- tree/LCA
def _build_tree():
    sizes = []
    n = S
    while True:
        sizes.append(n)
        if n == 1:
            break
        n = (n + 1) // 2
    offs = np.cumsum([0] + sizes)
    total = int(offs[-1])
    parent = np.arange(total)
    level = np.zeros(total, np.int32)
    for l, sz in enumerate(sizes):
        for i in range(sz):
            g = offs[l] + i
            level[g] = l
            if l + 1 < len(sizes):
                parent[g] = offs[l + 1] + i // 2
    L = len(sizes)
    chain = np.zeros((total, L), np.int64)
    for g in range(total):
        for l in range(L):
            if l < level[g]:
                chain[g, l] = -1 - g
            else:
                a = g
                while level[a] < l:
                    a = int(parent[a])
                chain[g, l] = a
    return sizes, parent.astype(np.int32), level, chain


SIZES, PARENT, LEVEL, CHAIN = _build_tree()
MAXL = len(SIZES) - 1
NTOT = PARENT.shape[0]


def _lca_matrix():
    eq = CHAIN[:, None, :] == CHAIN[None, :, :]
    first = np.argmax(eq, axis=-1)
    return CHAIN[np.arange(NTOT)[:, None], first].astype(np.int32)


LCA = _lca_matrix()


# ---------------------------------------------------------------- device program
_PROGRAM = None


def _build_program():
    nc = bass.Bass(trn_type="TRN2", num_swdge_queues=4)
    xs = nc.declare_dram_parameter("xs", [B * C, HC * W], F32, isOutput=False)
    mask = nc.declare_dram_parameter("mask", [B * C, B], F16, isOutput=False)
    seg_out = nc.declare_dram_parameter("seg_out", [B * C, S], F32, isOutput=True)
    # gray ships in the PSUM-native permutation [pixel%125, (half, j, b)];
    # the host undoes it (gray[b, (half*128+j)*125 + p]).
    gray_out = nc.declare_dram_parameter("gray_out", [125, 1024], F16, isOutput=True)

    # Raw-Bass program (no TileContext): the walrus build in this container
    # supports only ONE embedded sync-wait per DMA/matmul/drain instruction,
    # which Tile's auto-generated semaphores and kernel-tail drain violate
    # structurally. With explicit semaphores every instruction carries at
    # most one wait:
    #   Pool : 9 casting SWDGE DMAs (f32->f16), inc in_sem by 16 each
    #   PE   : per chunk, 1 wait on in_sem, then 32 "flipped" matmuls
    #          (stationary = 125-pixel slice, moving = [128,4] mask,
    #          out = [125 pixels, 4 batches]); 256 outputs fill exactly
    #          2 PSUM banks, inc pe_sem after each chunk
    #   DVE  : per chunk, 1 wait on in_sem, strip-sum tensor_reduce
    #   ACT  : waits pe_sem, evacuates each full PSUM bank (f32->f16),
    #          issues the gray HWDGE DMA itself
    #   SP   : waits dve_sem, issues the seg DMA
    PJ = 125                   # pixels per matmul (stationary free dim)
    JPC = R * W // PJ          # matmuls per chunk
    JPB = 512 // B             # matmul outputs per PSUM bank (128)
    CPB = JPB // JPC           # chunks per PSUM bank

    with ExitStack() as ctx:
        t16 = ctx.enter_context(nc.sbuf_tensor([B * C, HC * W], F16))
        mask_t = ctx.enter_context(nc.sbuf_tensor([B * C, B], F16))
        segbuf = ctx.enter_context(nc.sbuf_tensor([B * C, NCH * R * S], F32))
        seg_final = ctx.enter_context(nc.sbuf_tensor([B * C, S], F32))
        gbuf = ctx.enter_context(nc.sbuf_tensor([PJ, 2 * JPB * B], F16))
        ps0 = ctx.enter_context(nc.psum_tensor([B * C, JPB * B], F32))
        ps1 = ctx.enter_context(nc.psum_tensor([B * C, JPB * B], F32))
        in_sem = ctx.enter_context(nc.semaphore("in_sem"))
        pe_sem = ctx.enter_context(nc.semaphore("pe_sem"))
        dve_sem = ctx.enter_context(nc.semaphore("dve_sem"))
        g_sem = ctx.enter_context(nc.semaphore("g_sem"))
        s_sem = ctx.enter_context(nc.semaphore("s_sem"))
        block = ctx.enter_context(nc.Block(no_gpsimd_drain=True))
        banks = [ps0, ps1]

        @block.gpsimd
        def _(pool):
            pool.dma_start(mask_t[:], mask[:]).then_inc(in_sem, 16)
            for ch in range(NCH):
                sl = slice(ch * R * W, (ch + 1) * R * W)
                pool.dma_start(t16[:, sl], xs[:, sl]).then_inc(in_sem, 16)

        @block.tensor
        def _(pe):
            for ch in range(NCH):
                pe.wait_ge(in_sem, 16 * (ch + 2))
                for jj in range(JPC):
                    j = ch * JPC + jj
                    pst = banks[j // JPB]
                    slot = j % JPB
                    mm = pe.matmul(
                        pst[0:PJ, slot * B:(slot + 1) * B],
                        t16[:, j * PJ:(j + 1) * PJ],
                        mask_t[:],
                        start=True, stop=True,
                    )
                mm.then_inc(pe_sem, 1)

        @block.vector
        def _(dve):
            # stage 1: contiguous stride-50 walk, reduce w within each
            # (row, strip) -> [128, R*S] per chunk
            for ch in range(NCH):
                dve.wait_ge(in_sem, 16 * (ch + 2))
                v = t16[:, ch * R * W:(ch + 1) * R * W].rearrange(
                    "p (a w) -> p a w", a=R * S, w=SW
                )
                dve.tensor_reduce(
                    segbuf[:, ch * R * S:(ch + 1) * R * S], v,
                    axis=mybir.AxisListType.X, op=mybir.AluOpType.add,
                )
            # stage 2: fold rows -> [128, S]
            dve.tensor_reduce(
                seg_final[:],
                segbuf[:].rearrange("p (a s) -> p s a", a=NCH * R, s=S),
                axis=mybir.AxisListType.X, op=mybir.AluOpType.add,
            ).then_inc(dve_sem, 1)

        @block.scalar
        def _(act):
            for half in range(2):
                act.wait_ge(pe_sem, CPB * (half + 1))
                act.copy(
                    gbuf[:, half * JPB * B:(half + 1) * JPB * B],
                    banks[half][0:PJ, :],
                )
            act.dma_start(gray_out[:], gbuf[:]).then_inc(g_sem, 16)
            act.wait_ge(g_sem, 16)

        @block.sync
        def _(sp):
            sp.wait_ge(dve_sem, 1)
            sp.dma_start(seg_out[:], seg_final[:]).then_inc(s_sem, 16)
            sp.wait_ge(s_sem, 16)
    return nc


def _get_program():
    global _PROGRAM
    if _PROGRAM is None:
        _PROGRAM = _build_program()
    return _PROGRAM


def _make_mask():
    m = np.zeros((B * C, B), np.float16)
    m[np.arange(B * C), np.arange(B * C) // C] = 1.0 / C
    return m


def _run_device(x, **kwargs):
    nc = _get_program()
    mask_np = _make_mask()
    in_maps = []
    for k in range(NCORES):
        xs = np.ascontiguousarray(
            x[:, :, k * HC:(k + 1) * HC, :].reshape(B * C, HC * W)
        )
        in_maps.append({"xs": xs, "mask": mask_np})
    return run_bass_kernel_spmd(nc, in_maps, list(range(NCORES)), **kwargs)


def _finalize(seg, gray):
    """seg: [B*C, S] f64 total strip sums; gray: [B, H, W] f64 channel means."""
    nodes = (seg / (H * SW)).reshape(B, C, S).transpose(0, 2, 1)      # [B,S,C]
    texture = (gray ** 2).reshape(B, H, S, SW).sum(axis=(1, 3))      # [B,S]
    feats = nodes * (1.0 - TW) + texture[..., None] * TW             # [B,S,C]

    reg = 0.0
    cur = feats
    for sz in SIZES[1:]:
        n = cur.shape[1]
        ids = np.arange(n) // 2
        counts = np.bincount(ids, minlength=sz).astype(np.float64)
        summed = np.zeros((sz, B, C), np.float64)
        np.add.at(summed, ids, cur.transpose(1, 0, 2))
        cur = (summed / counts[:, None, None]).transpose(1, 0, 2)
        reg += (cur ** 2).mean()

    diff = feats[:, :, None, :] - feats[:, None, :, :]
    d = np.sqrt((diff ** 2).sum(-1) + 1e-12)                          # [B,S,S]
    w = 1.0 - LEVEL[LCA[:S, :S]].astype(np.float64) / MAXL
    loss = (w * d ** 2 + (1.0 - w) * np.maximum(MARGIN - d, 0.0) ** 2).mean() \
        + REG_W * reg

    tree = np.broadcast_to(
        np.stack([PARENT, LEVEL], axis=-1).astype(np.int32)[None], (B, NTOT, 2)
    ).copy()
    return tree, np.float32(loss)


def _gather(res):
    seg = np.zeros((B * C, S), np.float64)
    gray = np.zeros((B, H, W), np.float64)
    for k in range(NCORES):
        seg += res.results[k]["seg_out"].astype(np.float64)
        g = res.results[k]["gray_out"].astype(np.float64)      # [125, 1024]
        g = g.reshape(125, 2, 128, B).transpose(3, 1, 2, 0)    # [B, half, j, p]
        gray[:, k * HC:(k + 1) * HC, :] = g.reshape(B, HC, W)
    return seg, gray


def kernel(x):
    x = np.asarray(x, dtype=np.float32)
    res = _run_device(x)
    seg, gray = _gather(res)
    return _finalize(seg, gray)
